# revision 32
# baseline (speedup 1.0000x reference)
# Multi-head attention (B=2, N=4096, D=512, H=8) on 8 trn2 NeuronCores.
#
# Sharding: head-parallel (Megatron style). Core c computes head c for both
# batch elements: Q/K/V projections with the 64-row weight slices, full
# attention (scores, softmax, attn@V) for its 2 (batch, head) pairs, writes
# its [2, 4096, 4096] slice of the attention-probability output, and a
# row-parallel partial of the output projection. Host gathers: attn slices
# are concatenated, out partials are summed (+bo).
#
# The attn-probability output (1 GiB fp32) makes this memory-bound in
# principle, but on this box the PE throttles to half clock under full
# 8-core load and exp throughput (ScalarE, 1 elem/lane/cyc) is comparable,
# so the design balances PE / ACT / DMA:
#  - pass 1: scores computed transposed (S^T tiles) -> one exp -> feeds the
#    attn@V matmul as [V | 1]-stationary accumulation; the ones column
#    yields softmax row sums (Z) for free. ctx stays unnormalized; 1/Z is
#    applied per-partition at the output-projection evacuation.
#  - pass 2: scores recomputed in natural orientation -> second exp writes
#    f32 row-major attn straight into the staging buffer, normalized by a
#    per-partition DVE tensor_scalar, then 2 MB row-contiguous DMA stores.
#    (Recomputing scores beats PE-transposing exp tiles by a wide margin:
#    transposes pay per-128x128-tile LDWEIGHTS+drain and don't pipeline.)
#  - Q^T/K^T are stored duplicated in both partition halves so consecutive
#    matmuls alternate PE row groups, letting LDWEIGHTS overlap matmuls.
#  - emission order: pair-0 K/V, then per dma-tile Q + its attention
#    tiles, with pair-1 projection chunks woven between pair-0 attention
#    tiles (Tile is a trace scheduler: producers must be emitted before
#    consumers, so attention for a pair waits on that pair's full K/V).


import time
from contextlib import ExitStack

import numpy as np
import ml_dtypes

import concourse.bass as bass
import concourse.mybir as mybir
import concourse.tile as tile
from concourse import bacc
from concourse.bass import ds, ts

F32 = mybir.dt.float32
BF16 = mybir.dt.bfloat16
AF = mybir.ActivationFunctionType

HIDDEN = 512
HEADS = 8
DH = 64          # head dim
B = 2
N_FULL = 4096
N_CORES = 8

LAST_RESULT = None  # BassKernelResults of the most recent device run


def _build(n=N_FULL, td=1024, it_size=512):
    """Emit the per-core SPMD kernel. n = tokens per batch element."""
    nt = B * n                    # total tokens (batch-major)
    n_jc = n // 128               # key-token chunks per pair
    n_it = n // it_size           # query tiles per pair
    n_blk = nt // 128             # 128-token blocks, global
    td = min(td, n)

    nc = bacc.Bacc("TRN2", target_bir_lowering=False, debug=False,
                   num_devices=N_CORES)

    # ---- DRAM I/O ----
    xqT = nc.dram_tensor("xqT", [HIDDEN, nt], BF16, kind="ExternalInput")
    xkT = nc.dram_tensor("xkT", [HIDDEN, nt], BF16, kind="ExternalInput")
    xvT = nc.dram_tensor("xvT", [HIDDEN, nt], BF16, kind="ExternalInput")
    wqT = nc.dram_tensor("wqT", [HIDDEN, DH], BF16, kind="ExternalInput")
    wkT = nc.dram_tensor("wkT", [HIDDEN, DH], BF16, kind="ExternalInput")
    wvT = nc.dram_tensor("wvT", [HIDDEN, DH], BF16, kind="ExternalInput")
    bq = nc.dram_tensor("bq", [DH, 1], F32, kind="ExternalInput")
    bk = nc.dram_tensor("bk", [DH, 1], F32, kind="ExternalInput")
    bv = nc.dram_tensor("bv", [1, DH], BF16, kind="ExternalInput")
    woT = nc.dram_tensor("woT", [DH, HIDDEN], BF16, kind="ExternalInput")

    attn = nc.dram_tensor("attn", [B, n, n], F32, kind="ExternalOutput")
    opart = nc.dram_tensor("opart", [nt, HIDDEN], BF16, kind="ExternalOutput")

    with tile.TileContext(nc) as tc, ExitStack() as ctx:
        consts = ctx.enter_context(tc.tile_pool(name="consts", bufs=1))
        qk_pool = ctx.enter_context(tc.tile_pool(name="qk", bufs=1))
        v_pool = ctx.enter_context(tc.tile_pool(name="v", bufs=1))
        ctxT_pool = ctx.enter_context(tc.tile_pool(name="ctxT", bufs=1))
        recip_pool = ctx.enter_context(tc.tile_pool(name="recip", bufs=1))
        expt_pool = ctx.enter_context(tc.tile_pool(name="expt", bufs=12))
        astg_pool = ctx.enter_context(tc.tile_pool(name="astg", bufs=4))
        ostg_pool = ctx.enter_context(tc.tile_pool(name="ostg", bufs=2))
        misc_pool = ctx.enter_context(tc.tile_pool(name="misc", bufs=4))

        st_psum = ctx.enter_context(tc.tile_pool(name="stp", bufs=3, space="PSUM"))
        cx_psum = ctx.enter_context(tc.tile_pool(name="cxp", bufs=2, space="PSUM"))

        # ---- constants ----
        wq_sb = consts.tile([128, 4, DH], BF16, tag="wq")
        wk_sb = consts.tile([128, 4, DH], BF16, tag="wk")
        wv_sb = consts.tile([128, 4, DH], BF16, tag="wv")
        for w_sb, w_dram in ((wq_sb, wqT), (wk_sb, wkT), (wv_sb, wvT)):
            nc.gpsimd.dma_start(
                w_sb[:], w_dram[:].rearrange("(c p) d -> p c d", p=128))
        bq_sb = consts.tile([DH, 1], F32, tag="bq")
        bk_sb = consts.tile([DH, 1], F32, tag="bk")
        bv_sb = consts.tile([1, DH], BF16, tag="bv")
        ones_sb = consts.tile([1, 128], BF16, tag="ones")
        nc.gpsimd.memset(ones_sb[:], 1.0)
        onesf_sb = consts.tile([1, 1], F32, tag="onesf")
        nc.gpsimd.memset(onesf_sb[:], 1.0)
        nc.gpsimd.dma_start(bq_sb[:], bq[:])
        nc.gpsimd.dma_start(bk_sb[:], bk[:])
        nc.gpsimd.dma_start(bv_sb[:], bv[:])
        woT_sb = consts.tile([DH, HIDDEN], BF16, tag="wo")
        nc.gpsimd.dma_start(woT_sb[:], woT[:])

        # persistent activations
        # Q^T/K^T duplicated in both partition halves so consecutive
        # matmuls alternate PE row groups (lets LDWEIGHTS overlap matmuls)
        q_sb = qk_pool.tile([128, nt], BF16, tag="q")    # Q^T (scaled, +bias)
        k_sb = qk_pool.tile([128, nt], BF16, tag="k")    # K^T (+bias)
        # V blocks augmented with a ones column: [V_blk | 1] -> row sums
        v_sb = v_pool.tile([128, n_blk * (DH + 1)], BF16, tag="v")
        nc.gpsimd.memset(v_sb[:], 1.0)
        ctxT_sb = ctxT_pool.tile([DH, nt], BF16, tag="ctxT")
        recip_sb = recip_pool.tile([128, n_blk], F32, tag="recip")

        # ---- emission: K/V projections lead, Q and next-pair work woven
        # between attention tiles so ACT (the bottleneck) never idles.
        xin_pool = ctx.enter_context(tc.tile_pool(name="xin", bufs=3))
        w_nat = min(1024, n)  # natural-orientation scores tile width (keys)

        def proj_kv(p, di):
            sl = ds(p * n + di * td, td)
            xk = xin_pool.tile([128, 4, td], BF16, tag="xin")
            xv = xin_pool.tile([128, 4, td], BF16, tag="xin")
            for x_sb, x_dram in ((xk, xkT), (xv, xvT)):
                nc.gpsimd.dma_start(
                    x_sb[:],
                    x_dram[:].rearrange("(c p) t -> p c t", p=128)[:, :, sl])
            for tt in range(td // 512):
                tsl = ds(p * n + di * td + tt * 512, 512)
                ps = cx_psum.tile([DH, 512], F32, tag="cx")
                for ch in range(4):
                    nc.tensor.matmul(
                        ps[:], wk_sb[:, ch, :], xk[:, ch, ts(tt, 512)],
                        start=(ch == 0), stop=(ch == 3))
                nc.vector.tensor_scalar_add(k_sb[0:DH, tsl], ps[:], bk_sb[:])
                nc.vector.tensor_scalar_add(k_sb[DH:128, tsl], ps[:],
                                            bk_sb[:])
            for tb in range(td // 128):
                g = (p * n + di * td) // 128 + tb
                ps = cx_psum.tile([128, DH], F32, tag="cx")
                for ch in range(4):
                    nc.tensor.matmul(
                        ps[:], xv[:, ch, ts(tb, 128)], wv_sb[:, ch, :],
                        start=(ch == 0), stop=False)
                nc.tensor.matmul(ps[:], ones_sb[:], bv_sb[:],
                                 start=False, stop=True)
                nc.vector.tensor_copy(v_sb[:, ds(g * (DH + 1), DH)], ps[:])

        def proj_q(p, di):
            sl = ds(p * n + di * td, td)
            xq = xin_pool.tile([128, 4, td], BF16, tag="xin")
            nc.gpsimd.dma_start(
                xq[:], xqT[:].rearrange("(c p) t -> p c t", p=128)[:, :, sl])
            for tt in range(td // 512):
                tsl = ds(p * n + di * td + tt * 512, 512)
                ps = cx_psum.tile([DH, 512], F32, tag="cx")
                for ch in range(4):
                    nc.tensor.matmul(
                        ps[:], wq_sb[:, ch, :], xq[:, ch, ts(tt, 512)],
                        start=(ch == 0), stop=(ch == 3))
                nc.vector.tensor_scalar_add(q_sb[0:DH, tsl], ps[:], bq_sb[:])
                nc.vector.tensor_scalar_add(q_sb[DH:128, tsl], ps[:],
                                            bq_sb[:])

        def attn_p1_part(p, it, jc_lo, jc_hi, cx):
            # pass 1 (jc range): transposed scores + exp -> expT feeds ctx
            i0 = it * it_size
            for jc2 in range(jc_lo // 2, jc_hi // 2):
                sp = st_psum.tile([128, 2 * it_size], F32, tag="st")
                for h in range(2):
                    jc = jc2 * 2 + h
                    rg = ds(h * DH, DH)  # alternate PE row groups
                    nc.tensor.matmul(
                        sp[:, ts(h, it_size)],
                        k_sb[rg, ds(p * n + jc * 128, 128)],
                        q_sb[rg, ds(p * n + i0, it_size)],
                        start=True, stop=True)
                et = expt_pool.tile([128, 2 * it_size], BF16, tag="et")
                nc.scalar.activation(et[:], sp[:], AF.Exp)
                for h in range(2):
                    jc = jc2 * 2 + h
                    nc.tensor.matmul(
                        cx[:],
                        v_sb[:, ds((p * n_jc + jc) * (DH + 1), DH + 1)],
                        et[:, ts(h, it_size)],
                        start=(jc == 0), stop=(jc == n_jc - 1))

        def attn_finish(p, it, cx):
            i0 = it * it_size
            nc.vector.tensor_copy(ctxT_sb[:, ds(p * n + i0, it_size)],
                                  cx[0:DH, :])
            # rowsums (psum row 64) -> per-partition reciprocals
            z_row = misc_pool.tile([1, it_size], F32, tag="zrow")
            nc.vector.tensor_copy(z_row[:], cx[DH:DH + 1, :])
            zc = cx_psum.tile([128, it_size // 128], F32, tag="cx")
            for s in range(it_size // 128):
                nc.tensor.matmul(zc[:, ds(s, 1)],
                                 z_row[:, ts(s, 128)], onesf_sb[:],
                                 start=True, stop=True)
            blk0 = p * (n // 128) + it * (it_size // 128)
            nc.vector.reciprocal(recip_sb[:, ds(blk0, it_size // 128)], zc[:])

            og = ostg_pool.tile([128, it_size // 128, HIDDEN], BF16, tag="og")
            for s in range(it_size // 128):
                blk = blk0 + s
                # pass 2: natural-orientation scores + exp
                astg = astg_pool.tile([128, n], F32, tag="astg")
                for jt in range(n // w_nat):
                    nat = st_psum.tile([128, w_nat], F32, tag="st")
                    for h in range(w_nat // 512):
                        rg = ds(h * DH, DH)  # alternate PE row groups
                        nc.tensor.matmul(
                            nat[:, ts(h, 512)],
                            q_sb[rg, ds(p * n + i0 + s * 128, 128)],
                            k_sb[rg, ds(p * n + jt * w_nat + h * 512, 512)],
                            start=True, stop=True)
                    nc.scalar.activation(astg[:, ts(jt, w_nat)], nat[:],
                                         AF.Exp)
                nc.vector.tensor_scalar_mul(astg[:], astg[:],
                                            recip_sb[:, ds(blk, 1)])
                nc.sync.dma_start(attn[p, ds(i0 + s * 128, 128), :], astg[:])

                # output projection partial for this token block
                op = st_psum.tile([128, HIDDEN], F32, tag="st")
                nc.tensor.matmul(op[:], ctxT_sb[:, ds(blk * 128, 128)],
                                 woT_sb[:], start=True, stop=True)
                nc.vector.tensor_scalar_mul(og[:, s, :], op[:],
                                            recip_sb[:, ds(blk, 1)])
            nc.sync.dma_start(
                opart[:].rearrange("(g p) d -> p g d", p=128)
                [:, ds(blk0, it_size // 128), :],
                og[:])

        def attn_tile(p, it):
            cx = cx_psum.tile([DH + 1, it_size], F32, tag="cx")
            attn_p1_part(p, it, 0, n_jc, cx)
            attn_finish(p, it, cx)

        # Emission order: producers strictly before consumers (Tile is a
        # trace scheduler — a consumer emitted before its producer reads
        # stale data). Pass 1 of the first attention tile is split so exp
        # starts after only half of pair-0's K/V has landed; pair-1
        # projection chunks are woven between pair-0 attention tiles.
        ndt = n // td
        ipd = n_it // ndt  # attention i-tiles per dma-tile of tokens
        jc_pd = td // 128  # key chunks per dma-tile
        if ndt >= 2:
            proj_kv(0, 0)
            proj_kv(0, 1)
            proj_q(0, 0)
            cx0 = cx_psum.tile([DH + 1, it_size], F32, tag="cx")
            attn_p1_part(0, 0, 0, 2 * jc_pd, cx0)
            for di in range(2, ndt):
                proj_kv(0, di)
            attn_p1_part(0, 0, 2 * jc_pd, n_jc, cx0)
            attn_finish(0, 0, cx0)
            q0_done = 1
            start_it = 1
        else:
            for di in range(ndt):
                proj_kv(0, di)
            q0_done = 0
            start_it = 0
        p1_chunks = [lambda di=di: proj_kv(1, di) for di in range(ndt)]
        p1_chunks += [lambda di=di: proj_q(1, di) for di in range(ndt)]
        ci = 0
        for di in range(ndt):
            if di >= q0_done:
                proj_q(0, di)
            for it in range(di * ipd, (di + 1) * ipd):
                if it < start_it:
                    continue
                attn_tile(0, it)
                if ci < len(p1_chunks):
                    p1_chunks[ci]()
                    ci += 1
        while ci < len(p1_chunks):
            p1_chunks[ci]()
            ci += 1
        for it in range(n_it):
            attn_tile(1, it)

    nc.compile()
    return nc


_NC_CACHE = {}


def _get_nc(n=N_FULL):
    if n not in _NC_CACHE:
        _NC_CACHE[n] = _build(n=n)
    return _NC_CACHE[n]


def _prep_shared(query, key, value, n):
    nt = B * n
    outs = []
    for x in (query, key, value):
        xt = np.ascontiguousarray(
            np.asarray(x, np.float32).reshape(nt, HIDDEN).T
        ).astype(ml_dtypes.bfloat16)
        outs.append(xt)
    return outs


def _prep_core(c, Wq, bq, Wk, bk, Wv, bv, Wo):
    scale = 1.0 / np.sqrt(DH)
    sl = slice(c * DH, (c + 1) * DH)
    m = {}
    m["wqT"] = np.ascontiguousarray(
        (np.asarray(Wq, np.float32)[sl].T * scale)).astype(ml_dtypes.bfloat16)
    m["wkT"] = np.ascontiguousarray(
        np.asarray(Wk, np.float32)[sl].T).astype(ml_dtypes.bfloat16)
    m["wvT"] = np.ascontiguousarray(
        np.asarray(Wv, np.float32)[sl].T).astype(ml_dtypes.bfloat16)
    m["bq"] = (np.asarray(bq, np.float32)[sl] * scale).reshape(DH, 1).copy()
    m["bk"] = np.asarray(bk, np.float32)[sl].reshape(DH, 1).copy()
    m["bv"] = np.asarray(bv, np.float32)[sl].reshape(1, DH).astype(
        ml_dtypes.bfloat16)
    m["woT"] = np.ascontiguousarray(
        np.asarray(Wo, np.float32)[:, sl].T).astype(ml_dtypes.bfloat16)
    return m


def _enable_axon_trace():
    """Register the NTFF profile hook that this image's antenv lacks, so
    run_bass_kernel_spmd(trace=True) can capture device profiles."""
    import sys
    import types

    import concourse.bass_utils as bu
    from trn_agent_boot.trn_boot import _ntff_profile_via_ctypes

    hook = _ntff_profile_via_ctypes("/opt/axon/libaxon_pjrt.so")
    mod = types.ModuleType("antenv.axon_hooks")
    mod._hook = hook
    mod.set_axon_ntff_profile_hook = lambda h: setattr(mod, "_hook", h)
    mod.get_axon_ntff_profile_hook = lambda: mod._hook
    sys.modules["antenv.axon_hooks"] = mod
    # no artifact bucket in this container; keep artifacts local
    bu.upload_artifacts = lambda tmpdir: tmpdir


def kernel(query, key, value, Wq, bq, Wk, bk, Wv, bv, Wo, bo, *, n=N_FULL,
           trace=False):
    global LAST_RESULT
    from concourse.bass_utils import run_bass_kernel_spmd

    if trace:
        _enable_axon_trace()
    nc = _get_nc(n)
    xqT, xkT, xvT = _prep_shared(query, key, value, n)
    in_maps = []
    for c in range(N_CORES):
        m = _prep_core(c, Wq, bq, Wk, bk, Wv, bv, Wo)
        m["xqT"], m["xkT"], m["xvT"] = xqT, xkT, xvT
        in_maps.append(m)

    res = None
    for attempt in range(3):
        try:
            res = run_bass_kernel_spmd(nc, in_maps,
                                       core_ids=list(range(N_CORES)),
                                       trace=trace)
            break
        except Exception:
            if attempt == 2:
                raise
            time.sleep(5)
    LAST_RESULT = res

    attn_full = np.empty((B * HEADS, n, n), np.float32)
    out = np.zeros((B * n, HIDDEN), np.float32)
    for c in range(N_CORES):
        r = res.results[c]
        for b in range(B):
            attn_full[b * HEADS + c] = r["attn"][b]
        out += np.asarray(r["opart"], np.float32)
    out += np.asarray(bo, np.float32)
    return out.reshape(B, n, HIDDEN), attn_full


# revision 33
# speedup vs baseline: 1.0232x; 1.0232x over previous
# Multi-head attention (B=2, N=4096, D=512, H=8) on 8 trn2 NeuronCores.
#
# Sharding: head-parallel (Megatron style). Core c computes head c for both
# batch elements: Q/K/V projections with the 64-row weight slices, full
# attention (scores, softmax, attn@V) for its 2 (batch, head) pairs, writes
# its [2, 4096, 4096] slice of the attention-probability output, and a
# row-parallel partial of the output projection. Host gathers: attn slices
# are concatenated, out partials are summed (+bo).
#
# The attn-probability output (1 GiB fp32) makes this memory-bound in
# principle, but on this box the PE throttles to half clock under full
# 8-core load and exp throughput (ScalarE, 1 elem/lane/cyc) is comparable,
# so the design balances PE / ACT / DMA:
#  - pass 1: scores computed transposed (S^T tiles) -> one exp -> feeds the
#    attn@V matmul as [V | 1]-stationary accumulation; the ones column
#    yields softmax row sums (Z) for free. ctx stays unnormalized; 1/Z is
#    applied per-partition at the output-projection evacuation.
#  - pass 2: scores recomputed in natural orientation -> second exp writes
#    f32 row-major attn straight into the staging buffer, normalized by a
#    per-partition DVE tensor_scalar, then 2 MB row-contiguous DMA stores.
#    (Recomputing scores beats PE-transposing exp tiles by a wide margin:
#    transposes pay per-128x128-tile LDWEIGHTS+drain and don't pipeline.)
#  - Q^T/K^T are stored duplicated in both partition halves so consecutive
#    matmuls alternate PE row groups, letting LDWEIGHTS overlap matmuls.
#  - emission order: pair-0 K/V, then per dma-tile Q + its attention
#    tiles, with pair-1 projection chunks woven between pair-0 attention
#    tiles (Tile is a trace scheduler: producers must be emitted before
#    consumers, so attention for a pair waits on that pair's full K/V).


import time
from contextlib import ExitStack

import numpy as np
import ml_dtypes

import concourse.bass as bass
import concourse.mybir as mybir
import concourse.tile as tile
from concourse import bacc
from concourse.bass import ds, ts

F32 = mybir.dt.float32
BF16 = mybir.dt.bfloat16
AF = mybir.ActivationFunctionType

HIDDEN = 512
HEADS = 8
DH = 64          # head dim
B = 2
N_FULL = 4096
N_CORES = 8

LAST_RESULT = None  # BassKernelResults of the most recent device run


def _build(n=N_FULL, td=1024, it_size=512):
    """Emit the per-core SPMD kernel. n = tokens per batch element."""
    nt = B * n                    # total tokens (batch-major)
    n_jc = n // 128               # key-token chunks per pair
    n_it = n // it_size           # query tiles per pair
    n_blk = nt // 128             # 128-token blocks, global
    td = min(td, n)

    nc = bacc.Bacc("TRN2", target_bir_lowering=False, debug=False,
                   num_devices=N_CORES)

    # ---- DRAM I/O ----
    xqT = nc.dram_tensor("xqT", [HIDDEN, nt], BF16, kind="ExternalInput")
    xkT = nc.dram_tensor("xkT", [HIDDEN, nt], BF16, kind="ExternalInput")
    xvT = nc.dram_tensor("xvT", [HIDDEN, nt], BF16, kind="ExternalInput")
    wqT = nc.dram_tensor("wqT", [HIDDEN, DH], BF16, kind="ExternalInput")
    wkT = nc.dram_tensor("wkT", [HIDDEN, DH], BF16, kind="ExternalInput")
    wvT = nc.dram_tensor("wvT", [HIDDEN, DH], BF16, kind="ExternalInput")
    bq = nc.dram_tensor("bq", [DH, 1], F32, kind="ExternalInput")
    bk = nc.dram_tensor("bk", [DH, 1], F32, kind="ExternalInput")
    bv = nc.dram_tensor("bv", [1, DH], BF16, kind="ExternalInput")
    woT = nc.dram_tensor("woT", [DH, HIDDEN], BF16, kind="ExternalInput")

    attn = nc.dram_tensor("attn", [B, n, n], F32, kind="ExternalOutput")
    opart = nc.dram_tensor("opart", [nt, HIDDEN], BF16, kind="ExternalOutput")

    with tile.TileContext(nc) as tc, ExitStack() as ctx:
        consts = ctx.enter_context(tc.tile_pool(name="consts", bufs=1))
        qk_pool = ctx.enter_context(tc.tile_pool(name="qk", bufs=1))
        v_pool = ctx.enter_context(tc.tile_pool(name="v", bufs=1))
        ctxT_pool = ctx.enter_context(tc.tile_pool(name="ctxT", bufs=1))
        recip_pool = ctx.enter_context(tc.tile_pool(name="recip", bufs=1))
        expt_pool = ctx.enter_context(tc.tile_pool(name="expt", bufs=12))
        astg_pool = ctx.enter_context(tc.tile_pool(name="astg", bufs=4))
        ostg_pool = ctx.enter_context(tc.tile_pool(name="ostg", bufs=2))
        misc_pool = ctx.enter_context(tc.tile_pool(name="misc", bufs=4))

        st_psum = ctx.enter_context(tc.tile_pool(name="stp", bufs=3, space="PSUM"))
        cx_psum = ctx.enter_context(tc.tile_pool(name="cxp", bufs=2, space="PSUM"))

        # ---- constants ----
        wq_sb = consts.tile([128, 4, DH], BF16, tag="wq")
        wk_sb = consts.tile([128, 4, DH], BF16, tag="wk")
        wv_sb = consts.tile([128, 4, DH], BF16, tag="wv")
        for w_sb, w_dram in ((wq_sb, wqT), (wk_sb, wkT), (wv_sb, wvT)):
            nc.gpsimd.dma_start(
                w_sb[:], w_dram[:].rearrange("(c p) d -> p c d", p=128))
        bq_sb = consts.tile([DH, 1], F32, tag="bq")
        bk_sb = consts.tile([DH, 1], F32, tag="bk")
        bv_sb = consts.tile([1, DH], BF16, tag="bv")
        ones_sb = consts.tile([1, 128], BF16, tag="ones")
        nc.gpsimd.memset(ones_sb[:], 1.0)
        onesf_sb = consts.tile([1, 1], F32, tag="onesf")
        nc.gpsimd.memset(onesf_sb[:], 1.0)
        nc.gpsimd.dma_start(bq_sb[:], bq[:])
        nc.gpsimd.dma_start(bk_sb[:], bk[:])
        nc.gpsimd.dma_start(bv_sb[:], bv[:])
        woT_sb = consts.tile([DH, HIDDEN], BF16, tag="wo")
        nc.gpsimd.dma_start(woT_sb[:], woT[:])

        # persistent activations
        # Q^T/K^T duplicated in both partition halves so consecutive
        # matmuls alternate PE row groups (lets LDWEIGHTS overlap matmuls)
        q_sb = qk_pool.tile([128, nt], BF16, tag="q")    # Q^T (scaled, +bias)
        k_sb = qk_pool.tile([128, nt], BF16, tag="k")    # K^T (+bias)
        # V blocks augmented with a ones column: [V_blk | 1] -> row sums
        v_sb = v_pool.tile([128, n_blk * (DH + 1)], BF16, tag="v")
        nc.gpsimd.memset(v_sb[:], 1.0)
        ctxT_sb = ctxT_pool.tile([DH, nt], BF16, tag="ctxT")
        recip_sb = recip_pool.tile([128, n_blk], F32, tag="recip")

        # ---- emission: K/V projections lead, Q and next-pair work woven
        # between attention tiles so ACT (the bottleneck) never idles.
        xin_pool = ctx.enter_context(tc.tile_pool(name="xin", bufs=3))
        w_nat = min(1024, n)  # natural-orientation scores tile width (keys)

        def proj_kv(p, di):
            sl = ds(p * n + di * td, td)
            xk = xin_pool.tile([128, 4, td], BF16, tag="xin")
            xv = xin_pool.tile([128, 4, td], BF16, tag="xin")
            for x_sb, x_dram in ((xk, xkT), (xv, xvT)):
                nc.sync.dma_start(
                    x_sb[:],
                    x_dram[:].rearrange("(c p) t -> p c t", p=128)[:, :, sl])
            for tt in range(td // 512):
                tsl = ds(p * n + di * td + tt * 512, 512)
                ps = cx_psum.tile([DH, 512], F32, tag="cx")
                for ch in range(4):
                    nc.tensor.matmul(
                        ps[:], wk_sb[:, ch, :], xk[:, ch, ts(tt, 512)],
                        start=(ch == 0), stop=(ch == 3))
                nc.vector.tensor_scalar_add(k_sb[0:DH, tsl], ps[:], bk_sb[:])
                nc.vector.tensor_scalar_add(k_sb[DH:128, tsl], ps[:],
                                            bk_sb[:])
            for tb in range(td // 128):
                g = (p * n + di * td) // 128 + tb
                ps = cx_psum.tile([128, DH], F32, tag="cx")
                for ch in range(4):
                    nc.tensor.matmul(
                        ps[:], xv[:, ch, ts(tb, 128)], wv_sb[:, ch, :],
                        start=(ch == 0), stop=False)
                nc.tensor.matmul(ps[:], ones_sb[:], bv_sb[:],
                                 start=False, stop=True)
                nc.vector.tensor_copy(v_sb[:, ds(g * (DH + 1), DH)], ps[:])

        def proj_q(p, di):
            sl = ds(p * n + di * td, td)
            xq = xin_pool.tile([128, 4, td], BF16, tag="xin")
            nc.sync.dma_start(
                xq[:], xqT[:].rearrange("(c p) t -> p c t", p=128)[:, :, sl])
            for tt in range(td // 512):
                tsl = ds(p * n + di * td + tt * 512, 512)
                ps = cx_psum.tile([DH, 512], F32, tag="cx")
                for ch in range(4):
                    nc.tensor.matmul(
                        ps[:], wq_sb[:, ch, :], xq[:, ch, ts(tt, 512)],
                        start=(ch == 0), stop=(ch == 3))
                nc.vector.tensor_scalar_add(q_sb[0:DH, tsl], ps[:], bq_sb[:])
                nc.vector.tensor_scalar_add(q_sb[DH:128, tsl], ps[:],
                                            bq_sb[:])

        def attn_p1_part(p, it, jc_lo, jc_hi, cx):
            # pass 1 (jc range): transposed scores + exp -> expT feeds ctx
            i0 = it * it_size
            for jc2 in range(jc_lo // 2, jc_hi // 2):
                sp = st_psum.tile([128, 2 * it_size], F32, tag="st")
                for h in range(2):
                    jc = jc2 * 2 + h
                    rg = ds(h * DH, DH)  # alternate PE row groups
                    nc.tensor.matmul(
                        sp[:, ts(h, it_size)],
                        k_sb[rg, ds(p * n + jc * 128, 128)],
                        q_sb[rg, ds(p * n + i0, it_size)],
                        start=True, stop=True)
                et = expt_pool.tile([128, 2 * it_size], BF16, tag="et")
                nc.scalar.activation(et[:], sp[:], AF.Exp)
                for h in range(2):
                    jc = jc2 * 2 + h
                    nc.tensor.matmul(
                        cx[:],
                        v_sb[:, ds((p * n_jc + jc) * (DH + 1), DH + 1)],
                        et[:, ts(h, it_size)],
                        start=(jc == 0), stop=(jc == n_jc - 1))

        def attn_finish(p, it, cx):
            i0 = it * it_size
            nc.vector.tensor_copy(ctxT_sb[:, ds(p * n + i0, it_size)],
                                  cx[0:DH, :])
            # rowsums (psum row 64) -> per-partition reciprocals
            z_row = misc_pool.tile([1, it_size], F32, tag="zrow")
            nc.vector.tensor_copy(z_row[:], cx[DH:DH + 1, :])
            zc = cx_psum.tile([128, it_size // 128], F32, tag="cx")
            for s in range(it_size // 128):
                nc.tensor.matmul(zc[:, ds(s, 1)],
                                 z_row[:, ts(s, 128)], onesf_sb[:],
                                 start=True, stop=True)
            blk0 = p * (n // 128) + it * (it_size // 128)
            nc.vector.reciprocal(recip_sb[:, ds(blk0, it_size // 128)], zc[:])

            og = ostg_pool.tile([128, it_size // 128, HIDDEN], BF16, tag="og")
            for s in range(it_size // 128):
                blk = blk0 + s
                # pass 2: natural-orientation scores + exp
                astg = astg_pool.tile([128, n], F32, tag="astg")
                for jt in range(n // w_nat):
                    nat = st_psum.tile([128, w_nat], F32, tag="st")
                    for h in range(w_nat // 512):
                        rg = ds(h * DH, DH)  # alternate PE row groups
                        nc.tensor.matmul(
                            nat[:, ts(h, 512)],
                            q_sb[rg, ds(p * n + i0 + s * 128, 128)],
                            k_sb[rg, ds(p * n + jt * w_nat + h * 512, 512)],
                            start=True, stop=True)
                    nc.scalar.activation(astg[:, ts(jt, w_nat)], nat[:],
                                         AF.Exp)
                nc.vector.tensor_scalar_mul(astg[:], astg[:],
                                            recip_sb[:, ds(blk, 1)])
                nc.sync.dma_start(attn[p, ds(i0 + s * 128, 128), :], astg[:])

                # output projection partial for this token block
                op = st_psum.tile([128, HIDDEN], F32, tag="st")
                nc.tensor.matmul(op[:], ctxT_sb[:, ds(blk * 128, 128)],
                                 woT_sb[:], start=True, stop=True)
                nc.vector.tensor_scalar_mul(og[:, s, :], op[:],
                                            recip_sb[:, ds(blk, 1)])
            nc.sync.dma_start(
                opart[:].rearrange("(g p) d -> p g d", p=128)
                [:, ds(blk0, it_size // 128), :],
                og[:])

        def attn_tile(p, it):
            cx = cx_psum.tile([DH + 1, it_size], F32, tag="cx")
            attn_p1_part(p, it, 0, n_jc, cx)
            attn_finish(p, it, cx)

        # Emission order: producers strictly before consumers (Tile is a
        # trace scheduler — a consumer emitted before its producer reads
        # stale data). Pass 1 of the first attention tile is split so exp
        # starts after only half of pair-0's K/V has landed; pair-1
        # projection chunks are woven between pair-0 attention tiles.
        ndt = n // td
        ipd = n_it // ndt  # attention i-tiles per dma-tile of tokens
        jc_pd = td // 128  # key chunks per dma-tile
        if ndt >= 2:
            proj_kv(0, 0)
            proj_kv(0, 1)
            proj_q(0, 0)
            cx0 = cx_psum.tile([DH + 1, it_size], F32, tag="cx")
            attn_p1_part(0, 0, 0, 2 * jc_pd, cx0)
            for di in range(2, ndt):
                proj_kv(0, di)
            attn_p1_part(0, 0, 2 * jc_pd, n_jc, cx0)
            attn_finish(0, 0, cx0)
            q0_done = 1
            start_it = 1
        else:
            for di in range(ndt):
                proj_kv(0, di)
            q0_done = 0
            start_it = 0
        p1_chunks = [lambda di=di: proj_kv(1, di) for di in range(ndt)]
        p1_chunks += [lambda di=di: proj_q(1, di) for di in range(ndt)]
        ci = 0
        for di in range(ndt):
            if di >= q0_done:
                proj_q(0, di)
            for it in range(di * ipd, (di + 1) * ipd):
                if it < start_it:
                    continue
                attn_tile(0, it)
                if ci < len(p1_chunks):
                    p1_chunks[ci]()
                    ci += 1
        while ci < len(p1_chunks):
            p1_chunks[ci]()
            ci += 1
        for it in range(n_it):
            attn_tile(1, it)

    nc.compile()
    return nc


_NC_CACHE = {}


def _get_nc(n=N_FULL):
    if n not in _NC_CACHE:
        _NC_CACHE[n] = _build(n=n)
    return _NC_CACHE[n]


def _prep_shared(query, key, value, n):
    nt = B * n
    outs = []
    for x in (query, key, value):
        xt = np.ascontiguousarray(
            np.asarray(x, np.float32).reshape(nt, HIDDEN).T
        ).astype(ml_dtypes.bfloat16)
        outs.append(xt)
    return outs


def _prep_core(c, Wq, bq, Wk, bk, Wv, bv, Wo):
    scale = 1.0 / np.sqrt(DH)
    sl = slice(c * DH, (c + 1) * DH)
    m = {}
    m["wqT"] = np.ascontiguousarray(
        (np.asarray(Wq, np.float32)[sl].T * scale)).astype(ml_dtypes.bfloat16)
    m["wkT"] = np.ascontiguousarray(
        np.asarray(Wk, np.float32)[sl].T).astype(ml_dtypes.bfloat16)
    m["wvT"] = np.ascontiguousarray(
        np.asarray(Wv, np.float32)[sl].T).astype(ml_dtypes.bfloat16)
    m["bq"] = (np.asarray(bq, np.float32)[sl] * scale).reshape(DH, 1).copy()
    m["bk"] = np.asarray(bk, np.float32)[sl].reshape(DH, 1).copy()
    m["bv"] = np.asarray(bv, np.float32)[sl].reshape(1, DH).astype(
        ml_dtypes.bfloat16)
    m["woT"] = np.ascontiguousarray(
        np.asarray(Wo, np.float32)[:, sl].T).astype(ml_dtypes.bfloat16)
    return m


def _enable_axon_trace():
    """Register the NTFF profile hook that this image's antenv lacks, so
    run_bass_kernel_spmd(trace=True) can capture device profiles."""
    import sys
    import types

    import concourse.bass_utils as bu
    from trn_agent_boot.trn_boot import _ntff_profile_via_ctypes

    hook = _ntff_profile_via_ctypes("/opt/axon/libaxon_pjrt.so")
    mod = types.ModuleType("antenv.axon_hooks")
    mod._hook = hook
    mod.set_axon_ntff_profile_hook = lambda h: setattr(mod, "_hook", h)
    mod.get_axon_ntff_profile_hook = lambda: mod._hook
    sys.modules["antenv.axon_hooks"] = mod
    # no artifact bucket in this container; keep artifacts local
    bu.upload_artifacts = lambda tmpdir: tmpdir


def kernel(query, key, value, Wq, bq, Wk, bk, Wv, bv, Wo, bo, *, n=N_FULL,
           trace=False):
    global LAST_RESULT
    from concourse.bass_utils import run_bass_kernel_spmd

    if trace:
        _enable_axon_trace()
    nc = _get_nc(n)
    xqT, xkT, xvT = _prep_shared(query, key, value, n)
    in_maps = []
    for c in range(N_CORES):
        m = _prep_core(c, Wq, bq, Wk, bk, Wv, bv, Wo)
        m["xqT"], m["xkT"], m["xvT"] = xqT, xkT, xvT
        in_maps.append(m)

    res = None
    for attempt in range(3):
        try:
            res = run_bass_kernel_spmd(nc, in_maps,
                                       core_ids=list(range(N_CORES)),
                                       trace=trace)
            break
        except Exception:
            if attempt == 2:
                raise
            time.sleep(5)
    LAST_RESULT = res

    attn_full = np.empty((B * HEADS, n, n), np.float32)
    out = np.zeros((B * n, HIDDEN), np.float32)
    for c in range(N_CORES):
        r = res.results[c]
        for b in range(B):
            attn_full[b * HEADS + c] = r["attn"][b]
        out += np.asarray(r["opart"], np.float32)
    out += np.asarray(bo, np.float32)
    return out.reshape(B, n, HIDDEN), attn_full
